# revision 1
# baseline (speedup 1.0000x reference)
"""Trainium2 Bass kernel for nn_AdditiveCouplingLayer.

y = x; y[:, 1::2] += MLP(x[:, 0::2])  with a 512->1024->1024->512 relu MLP.

Strategy: data-parallel over 8 NeuronCores (batch 65536 -> 8192/core),
weights replicated. The MLP's first two layers run in "transposed
activation" space (features on partitions, batch on the free dim) so
every matmul uses the natural weight layout; the host supplies the
masked half of x pre-transposed and pre-cast to fp16. Layer 3 swaps the
matmul operand roles (h2 slice stationary, W3 moving) so the
translation comes out in natural [batch, feature] layout — no output
transpose needed. Matmuls run in fp16 (1 cycle/row on the PE vs 4 for
fp32) with fp32 PSUM accumulation; weights are pre-cast to fp16 on the
host. MODE "f16x3" upgrades to near-fp32 precision via a 3-term hi/lo
split (3x the matmul work).
"""

import os
import sys

sys.path.insert(0, "/opt/trn_rl_repo")

import numpy as np

B, D, F, H = 65536, 1024, 512, 1024
NCORES = 8
BPC = B // NCORES  # rows per core
TB = 512  # batch tile (matmul free dim)
NBT = BPC // TB  # batch tiles per core
MODE = os.environ.get("BASS_COUPLING_MODE", "f16")

_cache = {}


def _build(mode):
    import concourse.bacc as bacc
    import concourse.tile as tile
    import concourse.mybir as mybir

    dt = mybir.dt
    AF = mybir.ActivationFunctionType
    split = mode == "f16x3"

    nc = bacc.Bacc(
        "TRN2", target_bir_lowering=False, debug=False, num_devices=NCORES
    )

    x_d = nc.dram_tensor("x", [BPC, D], dt.float32, kind="ExternalInput").ap()
    mT_d = nc.dram_tensor("mT", [F, BPC], dt.float16, kind="ExternalInput").ap()
    if split:
        mTl_d = nc.dram_tensor("mTl", [F, BPC], dt.float16, kind="ExternalInput").ap()
    w_d = {}
    for name, shape in (("w1", [F, H]), ("w2", [H, H]), ("w3", [H, F])):
        w_d[name] = nc.dram_tensor(name, shape, dt.float16, kind="ExternalInput").ap()
        if split:
            w_d[name + "l"] = nc.dram_tensor(
                name + "l", shape, dt.float16, kind="ExternalInput"
            ).ap()
    b1_d = nc.dram_tensor("b1m", [128, H // 128], dt.float32, kind="ExternalInput").ap()
    b2_d = nc.dram_tensor("b2m", [128, H // 128], dt.float32, kind="ExternalInput").ap()
    b3r_d = nc.dram_tensor("b3rep", [128, F], dt.float32, kind="ExternalInput").ap()
    y_d = nc.dram_tensor("y", [BPC, D], dt.float32, kind="ExternalOutput").ap()

    with tile.TileContext(nc) as tc:
        with (
            tc.tile_pool(name="wpool", bufs=1) as wpool,
            tc.tile_pool(name="xpool", bufs=3 if mode == "f16" else 2) as xpool,
            tc.tile_pool(name="mpool", bufs=3 if mode == "f16" else 2) as mpool,
            tc.tile_pool(name="hpool", bufs=3 if mode == "f16" else 2) as hpool,
            tc.tile_pool(name="pmm", bufs=6, space="PSUM") as pmm,
        ):
            # --- resident weights/biases ---
            deferred_w = []

            def load_w(name, rows, cols, eng):
                """One big tile + ONE DMA per weight matrix (k-chunks land
                side by side in the free dim); returns per-k column slices.
                eng=None defers the issue (pushed onto deferred_w; drained
                from the scalar engine between the first L1 evictions)."""
                nk = rows // 128
                big = wpool.tile(
                    [128, nk * cols], dt.float16, tag=name, name=name
                )

                def issue(eng):
                    eng.dma_start(
                        big[:].rearrange("p (k c) -> p k c", k=nk),
                        w_d[name].rearrange("(k p) c -> p k c", p=128),
                    )

                if eng is None:
                    deferred_w.append(issue)
                else:
                    issue(eng)
                return [big[:, k * cols : (k + 1) * cols] for k in range(nk)]

            def load_b(name, ap, n):
                # host pre-transposes biases to [128, n/128] so this DMA is
                # contiguous (a "(m p) -> p m" rearrange here is a 4-byte-
                # element gather that takes ~10us and stalls the DMA ring)
                t = wpool.tile([128, n // 128], dt.float32, tag=name)
                nc.scalar.dma_start(t[:], ap[:])
                return t

            # PE warmup: junk matmuls on a zeroed scratch tile keep the PE
            # busy through its HAM activity window while the first real
            # DMAs are in flight, so real matmuls start at 2.4GHz.
            scratch = wpool.tile([128, TB], dt.float16, tag="scratch")
            nc.gpsimd.memset(scratch[:], 0.0)
            pwarm = pmm.tile([128, TB], dt.float32, tag="warm", bufs=1)
            for _ in range(12):
                nc.tensor.matmul(
                    pwarm[:], scratch[:, :128], scratch[:], start=True, stop=True
                )

            # Startup DMA order is the critical path: W1 + tile-0 mT go
            # first on the sync queue; W2/W3 issue from the scalar queue
            # but only AFTER the first L1 evictions (drain_deferred), so
            # they don't steal HBM bandwidth from the W1/mT0 stream.
            # W1 is allocated here but its per-k-chunk DMAs are issued by
            # l1_tile(0) AFTER the tile-0 mT load, so the k-th matmul's
            # operands arrive progressively and the first real matmul can
            # start ~3us earlier than with one monolithic W1 transfer.
            w1big = wpool.tile([128, 4 * H], dt.float16, tag="w1")
            w1t = [w1big[:, k * H : (k + 1) * H] for k in range(4)]
            b1t = load_b("b1t", b1_d, H)
            b2t = load_b("b2t", b2_d, H)
            b3rep = wpool.tile([128, F], dt.float32, tag="b3rep")
            nc.scalar.dma_start(b3rep[:], b3r_d[:])
            w2t = load_w("w2", H, H, None)
            w3t = load_w("w3", H, F, None)
            if split:
                w1lbig = wpool.tile([128, 4 * H], dt.float16, tag="w1l")
                w1l = [w1lbig[:, k * H : (k + 1) * H] for k in range(4)]
                w2l = load_w("w2l", H, H, None)
                w3l = load_w("w3l", H, F, None)

            def mm_group(psum, pairs):
                n = len(pairs)
                for i, (lhsT, rhs) in enumerate(pairs):
                    nc.tensor.matmul(
                        psum[:], lhsT, rhs, start=(i == 0), stop=(i == n - 1)
                    )

            def layer(wt, wl, ins, ins_lo, bt, nout, oname, drain_deferred=False):
                """Transposed-space layer: out[m][feat128, TB] = relu(W.T@in + b)."""
                outs = []
                outs_lo = []
                nk = len(ins)
                for m in range(nout // 128):
                    p = pmm.tile([128, TB], dt.float32, tag="mm")
                    ms = slice(m * 128, (m + 1) * 128)
                    pairs = [(wt[k][:, ms], ins[k][:]) for k in range(nk)]
                    if split:
                        pairs += [(wt[k][:, ms], ins_lo[k][:]) for k in range(nk)]
                        pairs += [(wl[k][:, ms], ins[k][:]) for k in range(nk)]
                    mm_group(p, pairs)
                    o = hpool.tile([128, TB], dt.float16, tag=f"{oname}_{m}")
                    nc.scalar.activation(o[:], p[:], AF.Relu, bias=bt[:, m : m + 1])
                    if drain_deferred and deferred_w:
                        deferred_w.pop(0)(nc.scalar)
                    outs.append(o)
                    if split:
                        of = hpool.tile(
                            [128, TB], dt.float32, tag="hf_tmp", bufs=3
                        )
                        nc.scalar.activation(
                            of[:], p[:], AF.Relu, bias=bt[:, m : m + 1]
                        )
                        ol = hpool.tile([128, TB], dt.float16, tag=f"{oname}l_{m}")
                        nc.vector.tensor_sub(ol[:], of[:], o[:])
                        outs_lo.append(ol)
                return outs, outs_lo

            def l1_tile(bt_i):
                """mT loads + layer 1 for one batch tile (issued one tile
                ahead of layers 2/3 so the PE never stalls on the W2/W3
                arrival at startup, and mT is naturally prefetched)."""
                r0 = bt_i * TB

                def load_mt(dram, tag):
                    big = mpool.tile(
                        [128, 4 * TB], dt.float16, tag=tag, name=tag
                    )
                    nc.sync.dma_start(
                        big[:].rearrange("p (j c) -> p j c", j=4),
                        dram[:, r0 : r0 + TB].rearrange(
                            "(j p) c -> p j c", p=128
                        ),
                    )
                    return [
                        big[:, j * TB : (j + 1) * TB] for j in range(4)
                    ]

                mT = load_mt(mT_d, "mbig")
                mTl = load_mt(mTl_d, "mlbig") if split else []
                if bt_i == 0:
                    for k in range(4):
                        nc.sync.dma_start(
                            w1t[k], w_d["w1"][k * 128 : (k + 1) * 128, :]
                        )
                        if split:
                            nc.sync.dma_start(
                                w1l[k], w_d["w1l"][k * 128 : (k + 1) * 128, :]
                            )
                return layer(
                    w1t, w1l if split else None, mT, mTl, b1t, H, "h1",
                    drain_deferred=(bt_i == 0),
                )

            h1, h1l = l1_tile(0)
            pending_stores = []
            for bt_i in range(NBT):
                r0 = bt_i * TB

                h1_next = l1_tile(bt_i + 1) if bt_i + 1 < NBT else None

                # y stores ride the scalar HWDGE queue (the sync queue is
                # at ~its single-queue bandwidth limit with the loads),
                # deferred one iteration so the issue never blocks the ACT
                # sequencer waiting on the DVE adds.
                for rows, src in pending_stores:
                    nc.scalar.dma_start(rows, src)
                pending_stores = []

                # x tile (natural layout, needed only for the residual
                # assembly — issued after the mT loads on the same queue).
                # One 3-dim-AP DMA brings all 4 row-chunks side by side.
                xbig = xpool.tile([128, 4 * D], dt.float32, tag="xbig")
                nc.sync.dma_start(
                    xbig[:].rearrange("p (i c) -> p i c", i=4),
                    x_d[r0 : r0 + TB, :].rearrange("(i p) c -> p i c", p=128),
                )
                xb = [xbig[:, i * D : (i + 1) * D] for i in range(4)]
                h2, h2l = layer(
                    w2t, w2l if split else None, h1, h1l, b2t, H, "h2"
                )

                # y is assembled IN PLACE in the x tiles (even columns are
                # already x): odd cols += b3, then += translation.
                for i in range(4):
                    nc.vector.tensor_add(
                        xb[i][:, 1:D:2], xb[i][:, 1:D:2], b3rep[:]
                    )

                # layer 3 in natural layout: stationary = h2 batch-slice,
                # moving = W3 tile  ->  psum[batch128, F]
                for i in range(4):
                    p = pmm.tile([128, F], dt.float32, tag="mm")
                    bs = slice(i * 128, (i + 1) * 128)
                    pairs = [(h2[k][:, bs], w3t[k][:]) for k in range(8)]
                    if split:
                        pairs += [(h2l[k][:, bs], w3t[k][:]) for k in range(8)]
                        pairs += [(h2[k][:, bs], w3l[k][:]) for k in range(8)]
                    mm_group(p, pairs)
                    rows = y_d[r0 + i * 128 : r0 + (i + 1) * 128, :]
                    if bt_i == NBT - 1:
                        # final tile: split the add+store chain (quarters
                        # for the very last chunk, halves otherwise) and
                        # alternate store queues, so the kernel tail after
                        # the last matmul is as short as possible
                        nsp = 4 if i == 3 else 2
                        w = D // nsp
                        fw = F // nsp
                        for h in range(nsp):
                            osl = slice(h * w + 1, (h + 1) * w, 2)
                            nc.vector.tensor_add(
                                xb[i][:, osl], xb[i][:, osl],
                                p[:, h * fw : (h + 1) * fw],
                            )
                            eng = nc.sync if h % 2 == 0 else nc.scalar
                            eng.dma_start(
                                rows[:, h * w : (h + 1) * w],
                                xb[i][:, h * w : (h + 1) * w],
                            )
                    else:
                        nc.vector.tensor_add(
                            xb[i][:, 1:D:2], xb[i][:, 1:D:2], p[:]
                        )
                        pending_stores.append((rows[:], xb[i][:]))

                if h1_next is not None:
                    h1, h1l = h1_next

    nc.compile()
    return nc


def _get(mode):
    if mode not in _cache:
        _cache[mode] = _build(mode)
    return _cache[mode]


def _in_maps(x, W1, b1, W2, b2, W3, b3):
    split = MODE == "f16x3"

    def prep_w(w):
        hi = np.asarray(w, dtype=np.float32).astype(np.float16)
        if not split:
            return {"": hi}
        lo = (np.asarray(w, dtype=np.float32) - hi.astype(np.float32)).astype(
            np.float16
        )
        return {"": hi, "l": lo}

    ws = {}
    for name, w in (("w1", W1), ("w2", W2), ("w3", W3)):
        for suf, arr in prep_w(w).items():
            ws[name + suf] = arr

    common = dict(
        ws,
        b1m=np.ascontiguousarray(np.asarray(b1, np.float32).reshape(-1, 128).T),
        b2m=np.ascontiguousarray(np.asarray(b2, np.float32).reshape(-1, 128).T),
        b3rep=np.ascontiguousarray(
            np.broadcast_to(np.asarray(b3, np.float32), (128, F))
        ),
    )
    x = np.ascontiguousarray(np.asarray(x, np.float32))
    in_maps = []
    for c in range(NCORES):
        xs = x[c * BPC : (c + 1) * BPC]
        masked_t = np.ascontiguousarray(xs[:, 0::2].T)  # [F, BPC] f32
        m = dict(common, x=xs, mT=masked_t.astype(np.float16))
        if split:
            m["mTl"] = (masked_t - m["mT"].astype(np.float32)).astype(np.float16)
        in_maps.append(m)
    return in_maps


def kernel(x, W1, b1, W2, b2, W3, b3):
    from concourse.bass_utils import run_bass_kernel_spmd

    nc = _get(MODE)
    res = run_bass_kernel_spmd(
        nc, _in_maps(x, W1, b1, W2, b2, W3, b3), core_ids=list(range(NCORES))
    )
    return np.concatenate([res.results[c]["y"] for c in range(NCORES)], axis=0)



# revision 4
# speedup vs baseline: 1.8577x; 1.8577x over previous
"""Trainium2 Bass kernel for nn_AdditiveCouplingLayer.

y = x; y[:, 1::2] += MLP(x[:, 0::2])  with a 512->1024->1024->512 relu MLP.

Strategy: data-parallel over 8 NeuronCores (batch 65536 -> 8192/core),
weights replicated. The MLP's first two layers run in "transposed
activation" space (features on partitions, batch on the free dim) so
every matmul uses the natural weight layout; the host supplies the
masked half of x pre-transposed. Layer 3 swaps the matmul operand roles
(h2 slice stationary, W3 moving) so the translation comes out in
natural [batch, feature] layout — no output transpose needed.

All matmuls run in fp8 e4m3 with MatmulPerfMode.DoubleRow (2 PE rows
per cycle -> 2x the fp16 matmul throughput) and fp32 PSUM accumulation.
Weights are pre-scaled by 2048 on the host so their small entries
(std ~0.02) land in e4m3's normal range; the descale (exact 2^-11) is
folded into the scalar-engine activation for layers 1/2 and into the
DVE scalar_tensor_tensor for layer 3. b3 is pre-added into x's odd
columns on the host, so layer-3 assembly is a single fused
(psum * 1/s + x) DVE op. The output rel-err budget is dominated by x
itself (std 1) while the MLP translation is small (std ~0.1), so fp8's
~2% matmul error on the translation contributes only ~2e-3 overall.
"""

import os
import sys

sys.path.insert(0, "/opt/trn_rl_repo")

import numpy as np

B, D, F, H = 65536, 1024, 512, 1024
NCORES = 8
BPC = B // NCORES  # rows per core
TB = 512  # batch tile (matmul free dim)
NBT = BPC // TB  # batch tiles per core
WSCALE = 2048.0  # host-side weight pre-scale (power of 2: exact descale)

_cache = {}


def _build():
    import concourse.bacc as bacc
    import concourse.tile as tile
    import concourse.mybir as mybir

    dt = mybir.dt
    AF = mybir.ActivationFunctionType
    DR = mybir.MatmulPerfMode.DoubleRow
    ALU = mybir.AluOpType

    nc = bacc.Bacc(
        "TRN2", target_bir_lowering=False, debug=False, num_devices=NCORES
    )

    x_d = nc.dram_tensor("x", [BPC, D], dt.float32, kind="ExternalInput").ap()
    mT_d = nc.dram_tensor("mT", [F, BPC], dt.float8e4, kind="ExternalInput").ap()
    w_d = {}
    for name, shape in (("w1", [F, H]), ("w2", [H, H]), ("w3", [H, F])):
        w_d[name] = nc.dram_tensor(name, shape, dt.float8e4, kind="ExternalInput").ap()
    b1_d = nc.dram_tensor("b1m", [128, H // 128], dt.float32, kind="ExternalInput").ap()
    b2_d = nc.dram_tensor("b2m", [128, H // 128], dt.float32, kind="ExternalInput").ap()
    y_d = nc.dram_tensor("y", [BPC, D], dt.float32, kind="ExternalOutput").ap()

    with tile.TileContext(nc) as tc:
        with (
            tc.tile_pool(name="wpool", bufs=1) as wpool,
            tc.tile_pool(name="xpool", bufs=3) as xpool,
            tc.tile_pool(name="mpool", bufs=3) as mpool,
            tc.tile_pool(name="hpool", bufs=3) as hpool,
            tc.tile_pool(name="pmm", bufs=6, space="PSUM") as pmm,
        ):
            # --- resident weights/biases ---
            deferred_w = []

            def load_w(name, rows, cols, eng):
                """One big tile + ONE DMA per weight matrix (k-chunks land
                side by side in the free dim); returns the 3-dim
                [128, nk, cols] view for DoubleRow pair slicing.
                eng=None defers the issue (pushed onto deferred_w; drained
                from the scalar engine between the first L1 evictions)."""
                nk = rows // 128
                big = wpool.tile([128, nk * cols], dt.float8e4, tag=name, name=name)

                def issue(eng):
                    eng.dma_start(
                        big[:].rearrange("p (k c) -> p k c", k=nk),
                        w_d[name].rearrange("(k p) c -> p k c", p=128),
                    )

                if eng is None:
                    deferred_w.append(issue)
                else:
                    issue(eng)
                return big[:].rearrange("p (k c) -> p k c", k=nk)

            def load_b(name, ap, n):
                # host pre-transposes biases to [128, n/128] so this DMA is
                # contiguous (a "(m p) -> p m" rearrange here is a 4-byte-
                # element gather that takes ~10us and stalls the DMA ring)
                t = wpool.tile([128, n // 128], dt.float32, tag=name)
                nc.scalar.dma_start(t[:], ap[:])
                return t

            # PE warmup: junk matmuls on a zeroed scratch tile keep the PE
            # busy through its HAM activity window while the first real
            # DMAs are in flight, so real matmuls start at 2.4GHz.
            scratch = wpool.tile([128, TB], dt.float16, tag="scratch")
            nc.gpsimd.memset(scratch[:], 0.0)
            pwarm = pmm.tile([128, TB], dt.float32, tag="warm", bufs=1)
            for _ in range(12):
                nc.tensor.matmul(
                    pwarm[:], scratch[:, :128], scratch[:], start=True, stop=True
                )

            # Startup DMA order is the critical path: W1 + tile-0 mT go
            # first on the sync queue; W2/W3 issue from the scalar queue
            # but only AFTER the first L1 evictions (drain_deferred), so
            # they don't steal HBM bandwidth from the W1/mT0 stream.
            # W1 is allocated here but its per-k-chunk DMAs are issued by
            # l1_tile(0) AFTER the tile-0 mT load, so the k-th matmul's
            # operands arrive progressively and the first real matmul can
            # start earlier than with one monolithic W1 transfer.
            w1big = wpool.tile([128, 4 * H], dt.float8e4, tag="w1")
            w1r = w1big[:].rearrange("p (k c) -> p k c", k=4)
            b1t = load_b("b1t", b1_d, H)
            b2t = load_b("b2t", b2_d, H)
            w2r = load_w("w2", H, H, None)
            w3r = load_w("w3", H, F, None)

            def layer(wr, nkp, ins_r, bt, oname, drain_deferred=False):
                """Transposed-space fp8 layer: for each output 128-chunk m,
                out[:, m*TB:] = fp8(relu(psum * 1/WSCALE + b)).
                wr: [128, 2*nkp, cols] weight view; ins_r: [128, 2*nkp, TB]
                moving view. Returns the big output tile's 3-dim view."""
                obig = hpool.tile([128, 8 * TB], dt.float8e4, tag=oname, name=oname)
                for m in range(8):
                    p = pmm.tile([128, TB], dt.float32, tag="mm")
                    ms = slice(m * 128, (m + 1) * 128)
                    for kp in range(nkp):
                        nc.tensor.matmul(
                            p[:],
                            wr[:, 2 * kp : 2 * kp + 2, ms],
                            ins_r[:, 2 * kp : 2 * kp + 2, :],
                            start=(kp == 0),
                            stop=(kp == nkp - 1),
                            perf_mode=DR,
                        )
                    nc.scalar.activation(
                        obig[:, m * TB : (m + 1) * TB],
                        p[:],
                        AF.Relu,
                        bias=bt[:, m : m + 1],
                        scale=1.0 / WSCALE,
                    )
                    if drain_deferred and deferred_w:
                        deferred_w.pop(0)(nc.scalar)
                return obig[:].rearrange("p (k c) -> p k c", k=8)

            def l1_tile(bt_i):
                """mT load + layer 1 for one batch tile (issued one tile
                ahead of layers 2/3 so the PE never stalls on the W2/W3
                arrival at startup, and mT is naturally prefetched)."""
                r0 = bt_i * TB
                mbig = mpool.tile([128, 4 * TB], dt.float8e4, tag="mbig", name="mbig")
                nc.sync.dma_start(
                    mbig[:].rearrange("p (j c) -> p j c", j=4),
                    mT_d[:, r0 : r0 + TB].rearrange("(j p) c -> p j c", p=128),
                )
                if bt_i == 0:
                    for k in range(4):
                        nc.sync.dma_start(
                            w1big[:, k * H : (k + 1) * H],
                            w_d["w1"][k * 128 : (k + 1) * 128, :],
                        )
                mr = mbig[:].rearrange("p (j c) -> p j c", j=4)
                return layer(w1r, 2, mr, b1t, "h1", drain_deferred=(bt_i == 0))

            h1 = l1_tile(0)
            pending_stores = []
            for bt_i in range(NBT):
                r0 = bt_i * TB

                h1_next = l1_tile(bt_i + 1) if bt_i + 1 < NBT else None

                # y stores ride the scalar HWDGE queue (the sync queue is
                # at ~its single-queue bandwidth limit with the loads),
                # deferred one iteration so the issue never blocks the ACT
                # sequencer waiting on the DVE adds.
                for rows, src in pending_stores:
                    nc.scalar.dma_start(rows, src)
                pending_stores = []

                # x tile (natural layout, b3 pre-added to odd cols on the
                # host; needed only for the residual assembly — issued
                # after the mT loads on the same queue). One 3-dim-AP DMA
                # brings all 4 row-chunks side by side.
                xbig = xpool.tile([128, 4 * D], dt.float32, tag="xbig")
                nc.sync.dma_start(
                    xbig[:].rearrange("p (i c) -> p i c", i=4),
                    x_d[r0 : r0 + TB, :].rearrange("(i p) c -> p i c", p=128),
                )
                xb = [xbig[:, i * D : (i + 1) * D] for i in range(4)]
                h2 = layer(w2r, 4, h1, b2t, "h2")

                # layer 3 in natural layout: stationary = h2 batch-slice
                # pair, moving = W3 pair  ->  psum[batch128, F]; then one
                # fused DVE op: y_odd = psum * 1/WSCALE + x_odd.
                for i in range(4):
                    p = pmm.tile([128, F], dt.float32, tag="mm")
                    bs = slice(i * 128, (i + 1) * 128)
                    for kp in range(4):
                        nc.tensor.matmul(
                            p[:],
                            h2[:, 2 * kp : 2 * kp + 2, bs],
                            w3r[:, 2 * kp : 2 * kp + 2, :],
                            start=(kp == 0),
                            stop=(kp == 3),
                            perf_mode=DR,
                        )
                    rows = y_d[r0 + i * 128 : r0 + (i + 1) * 128, :]
                    if bt_i == NBT - 1:
                        # final tile: split the add+store chain (quarters
                        # for the very last chunk, halves otherwise) and
                        # alternate store queues, so the kernel tail after
                        # the last matmul is as short as possible
                        nsp = 4 if i == 3 else 2
                        w = D // nsp
                        fw = F // nsp
                        for h in range(nsp):
                            osl = slice(h * w + 1, (h + 1) * w, 2)
                            nc.vector.scalar_tensor_tensor(
                                xb[i][:, osl],
                                p[:, h * fw : (h + 1) * fw],
                                1.0 / WSCALE,
                                xb[i][:, osl],
                                ALU.mult,
                                ALU.add,
                            )
                            eng = nc.sync if h % 2 == 0 else nc.scalar
                            eng.dma_start(
                                rows[:, h * w : (h + 1) * w],
                                xb[i][:, h * w : (h + 1) * w],
                            )
                    else:
                        nc.vector.scalar_tensor_tensor(
                            xb[i][:, 1:D:2],
                            p[:],
                            1.0 / WSCALE,
                            xb[i][:, 1:D:2],
                            ALU.mult,
                            ALU.add,
                        )
                        pending_stores.append((rows[:], xb[i][:]))

                if h1_next is not None:
                    h1 = h1_next

    nc.compile()
    return nc


MODE = "fp8"  # single mode; kept for test.py compatibility


def _get(mode=None):
    if "nc" not in _cache:
        _cache["nc"] = _build()
    return _cache["nc"]


def _in_maps(x, W1, b1, W2, b2, W3, b3):
    import ml_dtypes

    f8 = ml_dtypes.float8_e4m3

    ws = {
        name: (np.asarray(w, np.float32) * WSCALE).astype(f8)
        for name, w in (("w1", W1), ("w2", W2), ("w3", W3))
    }

    common = dict(
        ws,
        b1m=np.ascontiguousarray(np.asarray(b1, np.float32).reshape(-1, 128).T),
        b2m=np.ascontiguousarray(np.asarray(b2, np.float32).reshape(-1, 128).T),
    )
    x = np.asarray(x, np.float32)
    xb3 = np.array(x, np.float32, copy=True)  # b3 pre-added to odd columns
    xb3[:, 1::2] += np.asarray(b3, np.float32)
    in_maps = []
    for c in range(NCORES):
        xs = xb3[c * BPC : (c + 1) * BPC]
        masked_t = np.ascontiguousarray(x[c * BPC : (c + 1) * BPC, 0::2].T)
        m = dict(common, x=xs, mT=masked_t.astype(f8))
        in_maps.append(m)
    return in_maps


def kernel(x, W1, b1, W2, b2, W3, b3):
    from concourse.bass_utils import run_bass_kernel_spmd

    nc = _get()
    res = run_bass_kernel_spmd(
        nc, _in_maps(x, W1, b1, W2, b2, W3, b3), core_ids=list(range(NCORES))
    )
    return np.concatenate([res.results[c]["y"] for c in range(NCORES)], axis=0)


# revision 9
# speedup vs baseline: 1.8781x; 1.0110x over previous
"""Trainium2 Bass kernel for nn_AdditiveCouplingLayer.

y = x; y[:, 1::2] += MLP(x[:, 0::2])  with a 512->1024->1024->512 relu MLP.

Strategy: data-parallel over 8 NeuronCores (batch 65536 -> 8192/core),
weights replicated. The MLP's first two layers run in "transposed
activation" space (features on partitions, batch on the free dim) so
every matmul uses the natural weight layout; layer 3 swaps the matmul
operand roles (h2 slice stationary, W3 moving) so the translation comes
out in natural [batch, feature] layout — no output transpose needed.

All matmuls run in fp8 e4m3 with MatmulPerfMode.DoubleRow (2 PE rows
per cycle -> 2x the fp16 matmul throughput) and fp32 PSUM accumulation.
Weights are pre-scaled by 2048 on the host so their small entries
(std ~0.02) land in e4m3's normal range; the descale (exact 2^-11) is
folded into the scalar-engine activation for layers 1/2 and into the
DVE scalar_tensor_tensor for layer 3. b3 is pre-added into x's odd
columns on the host, so layer-3 assembly is a single fused
(psum * 1/s + x) DVE op. The output rel-err budget is dominated by x
itself (std 1) while the MLP translation is small (std ~0.1), so fp8's
~2% matmul error on the translation contributes only ~4e-3 overall.

DMA layout: HWDGE queues generate descriptors at ~10ns each, so
throughput is descriptor-size-bound. The host pre-permutes every load
into its exact SBUF layout ([128 partitions, free]) so each transfer is
one DMA with 2-16KB contiguous per-partition descriptors: mT tiles
(2KB), x tiles (16KB), whole weight matrices (4-8KB). Loads ride the
sync queue, activations own the scalar sequencer, DVE does the layer-3
adds, and the y stores ride the otherwise-idle gpsimd SWDGE queue.
"""

import os
import sys

sys.path.insert(0, "/opt/trn_rl_repo")

import numpy as np

B, D, F, H = 65536, 1024, 512, 1024
NCORES = 8
BPC = B // NCORES  # rows per core
TB = 512  # batch tile (matmul free dim)
NBT = BPC // TB  # batch tiles per core
WSCALE = 2048.0  # host-side weight pre-scale (power of 2: exact descale)

_cache = {}


def _build():
    import concourse.bacc as bacc
    import concourse.tile as tile
    import concourse.mybir as mybir

    dt = mybir.dt
    AF = mybir.ActivationFunctionType
    DR = mybir.MatmulPerfMode.DoubleRow
    ALU = mybir.AluOpType

    nc = bacc.Bacc(
        "TRN2", target_bir_lowering=False, debug=False, num_devices=NCORES
    )

    # All inputs pre-permuted on host into SBUF layout: [128, free].
    x_d = nc.dram_tensor(
        "x", [128, NBT * 4 * D], dt.float32, kind="ExternalInput"
    ).ap()
    mT_d = nc.dram_tensor(
        "mT", [128, NBT * 4 * TB], dt.float8e4, kind="ExternalInput"
    ).ap()
    w_d = {
        "w1": nc.dram_tensor("w1", [128, 4 * H], dt.float8e4, kind="ExternalInput").ap(),
        "w2": nc.dram_tensor("w2", [128, 8 * H], dt.float8e4, kind="ExternalInput").ap(),
        "w3": nc.dram_tensor("w3", [128, 8 * F], dt.float8e4, kind="ExternalInput").ap(),
    }
    b1_d = nc.dram_tensor("b1m", [128, H // 128], dt.float32, kind="ExternalInput").ap()
    b2_d = nc.dram_tensor("b2m", [128, H // 128], dt.float32, kind="ExternalInput").ap()
    y_d = nc.dram_tensor("y", [BPC, D], dt.float32, kind="ExternalOutput").ap()

    with tile.TileContext(nc) as tc:
        with (
            tc.tile_pool(name="wpool", bufs=1) as wpool,
            tc.tile_pool(name="xpool", bufs=3) as xpool,
            tc.tile_pool(name="mpool", bufs=3) as mpool,
            tc.tile_pool(name="hpool", bufs=3) as hpool,
            tc.tile_pool(name="pmm", bufs=6, space="PSUM") as pmm,
        ):
            # --- resident weights/biases ---
            def load_w(name, nk, cols, eng):
                """One contiguous DMA per weight matrix (host pre-permuted
                to the SBUF layout); returns the [128, nk, cols] view for
                DoubleRow pair slicing."""
                big = wpool.tile([128, nk * cols], dt.float8e4, tag=name, name=name)
                eng.dma_start(big[:], w_d[name][:])
                return big[:].rearrange("p (k c) -> p k c", k=nk)

            def load_b(name, ap, n):
                t = wpool.tile([128, n // 128], dt.float32, tag=name)
                nc.scalar.dma_start(t[:], ap[:])
                return t

            # PE warmup: junk matmuls on a zeroed scratch tile keep the PE
            # busy (and its power-state activity window open) while the
            # first real DMAs are in flight, so real matmuls start at full
            # clock with all PE quadrants active.
            scratch = wpool.tile([128, TB], dt.float16, tag="scratch")
            nc.gpsimd.memset(scratch[:], 0.0)
            pwarm = pmm.tile([128, TB], dt.float32, tag="warm", bufs=1)
            for _ in range(14):
                nc.tensor.matmul(
                    pwarm[:], scratch[:, :128], scratch[:], start=True, stop=True
                )

            # Startup: mT tile 0 on the sync queue and W1 on the gpsimd
            # (SWDGE) queue run in parallel, so the first real matmul's
            # operands are ready right as the warmup matmuls finish.
            # W2/W3 follow on the scalar queue behind the (tiny) bias
            # loads.
            w1r = load_w("w1", 4, H, nc.gpsimd)
            b1t = load_b("b1t", b1_d, H)
            b2t = load_b("b2t", b2_d, H)
            w2r = load_w("w2", 8, H, nc.scalar)
            w3r = load_w("w3", 8, F, nc.scalar)

            def layer(wr, nkp, ins_r, bt, oname):
                """Transposed-space fp8 layer: for each output 128-chunk m,
                out[:, m*TB:] = fp8(relu(psum * 1/WSCALE + b))."""
                obig = hpool.tile([128, 8 * TB], dt.float8e4, tag=oname, name=oname)
                for m in range(8):
                    p = pmm.tile([128, TB], dt.float32, tag="mm")
                    ms = slice(m * 128, (m + 1) * 128)
                    for kp in range(nkp):
                        nc.tensor.matmul(
                            p[:],
                            wr[:, 2 * kp : 2 * kp + 2, ms],
                            ins_r[:, 2 * kp : 2 * kp + 2, :],
                            start=(kp == 0),
                            stop=(kp == nkp - 1),
                            perf_mode=DR,
                        )
                    nc.scalar.activation(
                        obig[:, m * TB : (m + 1) * TB],
                        p[:],
                        AF.Relu,
                        bias=bt[:, m : m + 1],
                        scale=1.0 / WSCALE,
                    )
                return obig[:].rearrange("p (k c) -> p k c", k=8)

            def l1_tile(bt_i):
                """mT load + layer 1 for one batch tile (issued one tile
                ahead of layers 2/3 so mT is naturally prefetched)."""
                mbig = mpool.tile([128, 4 * TB], dt.float8e4, tag="mbig", name="mbig")
                nc.sync.dma_start(
                    mbig[:], mT_d[:, bt_i * 4 * TB : (bt_i + 1) * 4 * TB]
                )
                mr = mbig[:].rearrange("p (j c) -> p j c", j=4)
                return layer(w1r, 2, mr, b1t, "h1")

            h1 = l1_tile(0)
            for bt_i in range(NBT):
                r0 = bt_i * TB

                h1_next = l1_tile(bt_i + 1) if bt_i + 1 < NBT else None

                # x tile (natural layout per 128-row chunk, b3 pre-added to
                # odd cols on the host, pre-permuted so this is one DMA of
                # 16KB descriptors).
                xbig = xpool.tile([128, 4 * D], dt.float32, tag="xbig")
                nc.sync.dma_start(
                    xbig[:], x_d[:, bt_i * 4 * D : (bt_i + 1) * 4 * D]
                )
                xb = [xbig[:, i * D : (i + 1) * D] for i in range(4)]
                h2 = layer(w2r, 4, h1, b2t, "h2")

                # layer 3 in natural layout: stationary = h2 batch-slice
                # pair, moving = W3 pair  ->  psum[batch128, F]; then one
                # fused DVE op per row-chunk: y_odd = psum * 1/WSCALE +
                # x_odd, with the store issued from the DVE queue right
                # behind it.
                for i in range(4):
                    p = pmm.tile([128, F], dt.float32, tag="mm")
                    bs = slice(i * 128, (i + 1) * 128)
                    for kp in range(4):
                        nc.tensor.matmul(
                            p[:],
                            h2[:, 2 * kp : 2 * kp + 2, bs],
                            w3r[:, 2 * kp : 2 * kp + 2, :],
                            start=(kp == 0),
                            stop=(kp == 3),
                            perf_mode=DR,
                        )
                    rows = y_d[r0 + i * 128 : r0 + (i + 1) * 128, :]
                    if bt_i == NBT - 1:
                        # final tile: split the add+store chain (quarters
                        # for the very last chunk, halves otherwise) and
                        # spread stores across the idle sync/scalar queues,
                        # so the kernel tail after the last matmul is as
                        # short as possible
                        nsp = 4 if i == 3 else 2
                        w = D // nsp
                        fw = F // nsp
                        for h in range(nsp):
                            osl = slice(h * w + 1, (h + 1) * w, 2)
                            nc.vector.scalar_tensor_tensor(
                                xb[i][:, osl],
                                p[:, h * fw : (h + 1) * fw],
                                1.0 / WSCALE,
                                xb[i][:, osl],
                                ALU.mult,
                                ALU.add,
                            )
                            eng = (nc.sync, nc.scalar, nc.gpsimd)[h % 3]
                            eng.dma_start(
                                rows[:, h * w : (h + 1) * w],
                                xb[i][:, h * w : (h + 1) * w],
                            )
                    else:
                        nc.vector.scalar_tensor_tensor(
                            xb[i][:, 1:D:2],
                            p[:],
                            1.0 / WSCALE,
                            xb[i][:, 1:D:2],
                            ALU.mult,
                            ALU.add,
                        )
                        nc.gpsimd.dma_start(rows[:], xb[i][:])

                if h1_next is not None:
                    h1 = h1_next

    nc.compile()
    return nc


MODE = "fp8"  # single mode; kept for test.py compatibility


def _get(mode=None):
    if "nc" not in _cache:
        _cache["nc"] = _build()
    return _cache["nc"]


def _in_maps(x, W1, b1, W2, b2, W3, b3):
    import ml_dtypes

    f8 = ml_dtypes.float8_e4m3

    def prep_w(w, nk, cols):
        # fp8-quantize (pre-scaled) and permute [nk*128, cols] into the
        # SBUF-resident layout [128, nk*cols]
        q = (np.asarray(w, np.float32) * WSCALE).astype(f8)
        return np.ascontiguousarray(
            q.reshape(nk, 128, cols).transpose(1, 0, 2).reshape(128, nk * cols)
        )

    common = dict(
        w1=prep_w(W1, 4, H),
        w2=prep_w(W2, 8, H),
        w3=prep_w(W3, 8, F),
        b1m=np.ascontiguousarray(np.asarray(b1, np.float32).reshape(-1, 128).T),
        b2m=np.ascontiguousarray(np.asarray(b2, np.float32).reshape(-1, 128).T),
    )
    x = np.asarray(x, np.float32)
    xb3 = np.array(x, np.float32, copy=True)  # b3 pre-added to odd columns
    xb3[:, 1::2] += np.asarray(b3, np.float32)
    in_maps = []
    for c in range(NCORES):
        sl = slice(c * BPC, (c + 1) * BPC)
        # x tiles in SBUF layout: [p, (t i c)] = x[t*TB + i*128 + p, c]
        xh = np.ascontiguousarray(
            xb3[sl]
            .reshape(NBT, 4, 128, D)
            .transpose(2, 0, 1, 3)
            .reshape(128, NBT * 4 * D)
        )
        # mT tiles in SBUF layout: [p, (t j c)] = x[t*TB + c, 2*(j*128+p)]
        mq = x[sl, 0::2].T.astype(f8)  # [F, BPC]
        mh = np.ascontiguousarray(
            mq.reshape(4, 128, NBT, TB)
            .transpose(1, 2, 0, 3)
            .reshape(128, NBT * 4 * TB)
        )
        in_maps.append(dict(common, x=xh, mT=mh))
    return in_maps


def kernel(x, W1, b1, W2, b2, W3, b3):
    from concourse.bass_utils import run_bass_kernel_spmd

    nc = _get()
    res = run_bass_kernel_spmd(
        nc, _in_maps(x, W1, b1, W2, b2, W3, b3), core_ids=list(range(NCORES))
    )
    return np.concatenate([res.results[c]["y"] for c in range(NCORES)], axis=0)


# revision 11
# speedup vs baseline: 1.9012x; 1.0123x over previous
"""Trainium2 Bass kernel for nn_AdditiveCouplingLayer.

y = x; y[:, 1::2] += MLP(x[:, 0::2])  with a 512->1024->1024->512 relu MLP.

Strategy: data-parallel over 8 NeuronCores (batch 65536 -> 8192/core),
weights replicated. The MLP's first two layers run in "transposed
activation" space (features on partitions, batch on the free dim) so
every matmul uses the natural weight layout; layer 3 swaps the matmul
operand roles (h2 slice stationary, W3 moving) so the translation comes
out in natural [batch, feature] layout — no output transpose needed.

All matmuls run in fp8 e4m3 with MatmulPerfMode.DoubleRow (2 PE rows
per cycle -> 2x the fp16 matmul throughput) and fp32 PSUM accumulation.
Weights are pre-scaled by 2048 on the host so their small entries
(std ~0.02) land in e4m3's normal range; the descale (exact 2^-11) is
folded into the scalar-engine activation for layers 1/2 and into the
DVE scalar_tensor_tensor for layer 3. b3 is pre-added into x's odd
columns on the host, so layer-3 assembly is a single fused
(psum * 1/s + x) DVE op. The output rel-err budget is dominated by x
itself (std 1) while the MLP translation is small (std ~0.1), so fp8's
~2% matmul error on the translation contributes only ~4e-3 overall.

DMA layout: HWDGE queues generate descriptors at ~10ns each, so
throughput is descriptor-size-bound. The host pre-permutes every load
into its exact SBUF layout ([128 partitions, free]) so each transfer is
one DMA with 2-16KB contiguous per-partition descriptors: mT tiles
(2KB), x tiles (16KB), whole weight matrices (4-8KB). Loads ride the
sync queue, activations own the scalar sequencer, DVE does the layer-3
adds, and the y stores ride the otherwise-idle gpsimd SWDGE queue.

The batch is cut into 15x512 + 2x256 tiles: the kernel tail after the
final matmul is the last tile's add+store chain, so the last tiles are
narrow, their adds run on DVE and Pool in parallel, and their full-row
stores (4KB descriptors) go to the sync/scalar queues which are idle by
then.
"""

import os
import sys

sys.path.insert(0, "/opt/trn_rl_repo")

import numpy as np

B, D, F, H = 65536, 1024, 512, 1024
NCORES = 8
BPC = B // NCORES  # rows per core
WIDTHS = [512] * 15 + [256, 256]  # batch tile widths (sum = BPC)
WSCALE = 2048.0  # host-side weight pre-scale (power of 2: exact descale)

assert sum(WIDTHS) == BPC

_cache = {}


def _build():
    import concourse.bacc as bacc
    import concourse.tile as tile
    import concourse.mybir as mybir

    dt = mybir.dt
    AF = mybir.ActivationFunctionType
    DR = mybir.MatmulPerfMode.DoubleRow
    ALU = mybir.AluOpType

    nc = bacc.Bacc(
        "TRN2", target_bir_lowering=False, debug=False, num_devices=NCORES
    )

    NT = len(WIDTHS)
    r0s = [sum(WIDTHS[:t]) for t in range(NT)]  # tile start rows
    moffs = [4 * r for r in r0s]  # mT free-dim offsets ([128, 4*w] per tile)

    # All inputs pre-permuted on host into SBUF layout: [128, free].
    x_d = nc.dram_tensor(
        "x", [128, (BPC // 128) * D], dt.float32, kind="ExternalInput"
    ).ap()
    mT_d = nc.dram_tensor(
        "mT", [128, 4 * BPC], dt.float8e4, kind="ExternalInput"
    ).ap()
    w_d = {
        "w1": nc.dram_tensor("w1", [128, 4 * H], dt.float8e4, kind="ExternalInput").ap(),
        "w2": nc.dram_tensor("w2", [128, 8 * H], dt.float8e4, kind="ExternalInput").ap(),
        "w3": nc.dram_tensor("w3", [128, 8 * F], dt.float8e4, kind="ExternalInput").ap(),
    }
    b1_d = nc.dram_tensor("b1m", [128, H // 128], dt.float32, kind="ExternalInput").ap()
    b2_d = nc.dram_tensor("b2m", [128, H // 128], dt.float32, kind="ExternalInput").ap()
    y_d = nc.dram_tensor("y", [BPC, D], dt.float32, kind="ExternalOutput").ap()

    with tile.TileContext(nc) as tc:
        with (
            tc.tile_pool(name="wpool", bufs=1) as wpool,
            tc.tile_pool(name="xpool", bufs=3) as xpool,
            tc.tile_pool(name="mpool", bufs=3) as mpool,
            tc.tile_pool(name="hpool", bufs=3) as hpool,
            tc.tile_pool(name="pmm", bufs=6, space="PSUM") as pmm,
        ):
            # --- resident weights/biases ---
            def load_w(name, nk, cols, eng):
                """One contiguous DMA per weight matrix (host pre-permuted
                to the SBUF layout); returns the [128, nk, cols] view for
                DoubleRow pair slicing."""
                big = wpool.tile([128, nk * cols], dt.float8e4, tag=name, name=name)
                eng.dma_start(big[:], w_d[name][:])
                return big[:].rearrange("p (k c) -> p k c", k=nk)

            def load_b(name, ap, n):
                t = wpool.tile([128, n // 128], dt.float32, tag=name)
                nc.scalar.dma_start(t[:], ap[:])
                return t

            # PE warmup: junk matmuls on a zeroed scratch tile keep the PE
            # busy (and its power-state activity window open) while the
            # first real DMAs are in flight, so real matmuls start at full
            # clock with all PE quadrants active.
            scratch = wpool.tile([128, 512], dt.float16, tag="scratch")
            nc.gpsimd.memset(scratch[:], 0.0)
            pwarm = pmm.tile([128, 512], dt.float32, tag="warm", bufs=1)
            for _ in range(13):
                nc.tensor.matmul(
                    pwarm[:], scratch[:, :128], scratch[:], start=True, stop=True
                )

            # Startup: mT tile 0 on the sync queue and W1 on the gpsimd
            # (SWDGE) queue run in parallel, so the first real matmul's
            # operands are ready right as the warmup matmuls finish.
            # W2/W3 follow on the scalar queue behind the (tiny) bias
            # loads.
            w1r = load_w("w1", 4, H, nc.gpsimd)
            b1t = load_b("b1t", b1_d, H)
            b2t = load_b("b2t", b2_d, H)
            w2r = load_w("w2", 8, H, nc.scalar)
            w3r = load_w("w3", 8, F, nc.scalar)

            def layer(wr, nkp, ins_r, bt, oname, w):
                """Transposed-space fp8 layer: for each output 128-chunk m,
                out[:, m*w:] = fp8(relu(psum * 1/WSCALE + b))."""
                obig = hpool.tile([128, 8 * w], dt.float8e4, tag=oname, name=oname)
                for m in range(8):
                    p = pmm.tile([128, 512], dt.float32, tag="mm")
                    ms = slice(m * 128, (m + 1) * 128)
                    for kp in range(nkp):
                        nc.tensor.matmul(
                            p[:, :w],
                            wr[:, 2 * kp : 2 * kp + 2, ms],
                            ins_r[:, 2 * kp : 2 * kp + 2, :],
                            start=(kp == 0),
                            stop=(kp == nkp - 1),
                            perf_mode=DR,
                        )
                    nc.scalar.activation(
                        obig[:, m * w : (m + 1) * w],
                        p[:, :w],
                        AF.Relu,
                        bias=bt[:, m : m + 1],
                        scale=1.0 / WSCALE,
                    )
                return obig[:].rearrange("p (k c) -> p k c", k=8)

            def l1_tile(t):
                """mT load + layer 1 for one batch tile (issued one tile
                ahead of layers 2/3 so mT is naturally prefetched)."""
                w = WIDTHS[t]
                mbig = mpool.tile([128, 4 * w], dt.float8e4, tag="mbig", name="mbig")
                nc.sync.dma_start(mbig[:], mT_d[:, moffs[t] : moffs[t] + 4 * w])
                mr = mbig[:].rearrange("p (j c) -> p j c", j=4)
                return layer(w1r, 2, mr, b1t, "h1", w)

            h1 = l1_tile(0)
            for t in range(NT):
                w = WIDTHS[t]
                r0 = r0s[t]
                nch = w // 128  # 128-row chunks in this tile
                last = t == NT - 1

                h1_next = l1_tile(t + 1) if t + 1 < NT else None

                # x tile (natural layout per 128-row chunk, b3 pre-added to
                # odd cols on the host, pre-permuted so this is one DMA of
                # 16KB descriptors).
                xbig = xpool.tile([128, nch * D], dt.float32, tag="xbig")
                co = (r0 // 128) * D
                nc.sync.dma_start(xbig[:], x_d[:, co : co + nch * D])
                xb = [xbig[:, i * D : (i + 1) * D] for i in range(nch)]
                h2 = layer(w2r, 4, h1, b2t, "h2", w)

                # layer 3 in natural layout: stationary = h2 batch-slice
                # pair, moving = W3 pair  ->  psum[batch128, F]; then one
                # fused op per row-chunk: y_odd = psum * 1/WSCALE + x_odd,
                # with the store issued right behind it.
                for i in range(nch):
                    p = pmm.tile([128, 512], dt.float32, tag="mm")
                    bs = slice(i * 128, (i + 1) * 128)
                    for kp in range(4):
                        nc.tensor.matmul(
                            p[:],
                            h2[:, 2 * kp : 2 * kp + 2, bs],
                            w3r[:, 2 * kp : 2 * kp + 2, :],
                            start=(kp == 0),
                            stop=(kp == 3),
                            perf_mode=DR,
                        )
                    rows = y_d[r0 + i * 128 : r0 + (i + 1) * 128, :]
                    if last:
                        # final tile: full-row stores go to the idle
                        # sync/scalar queues (Pool can't read PSUM, so the
                        # adds stay on DVE — only 2 for the narrow tile)
                        qeng = (nc.sync, nc.scalar)[i % 2]
                    else:
                        qeng = nc.gpsimd
                    nc.vector.scalar_tensor_tensor(
                        xb[i][:, 1:D:2],
                        p[:],
                        1.0 / WSCALE,
                        xb[i][:, 1:D:2],
                        ALU.mult,
                        ALU.add,
                    )
                    qeng.dma_start(rows[:], xb[i][:])

                if h1_next is not None:
                    h1 = h1_next

    nc.compile()
    return nc


MODE = "fp8"  # single mode; kept for test.py compatibility


def _get(mode=None):
    if "nc" not in _cache:
        _cache["nc"] = _build()
    return _cache["nc"]


def _in_maps(x, W1, b1, W2, b2, W3, b3):
    import ml_dtypes

    f8 = ml_dtypes.float8_e4m3

    def prep_w(w, nk, cols):
        # fp8-quantize (pre-scaled) and permute [nk*128, cols] into the
        # SBUF-resident layout [128, nk*cols]
        q = (np.asarray(w, np.float32) * WSCALE).astype(f8)
        return np.ascontiguousarray(
            q.reshape(nk, 128, cols).transpose(1, 0, 2).reshape(128, nk * cols)
        )

    common = dict(
        w1=prep_w(W1, 4, H),
        w2=prep_w(W2, 8, H),
        w3=prep_w(W3, 8, F),
        b1m=np.ascontiguousarray(np.asarray(b1, np.float32).reshape(-1, 128).T),
        b2m=np.ascontiguousarray(np.asarray(b2, np.float32).reshape(-1, 128).T),
    )
    x = np.asarray(x, np.float32)
    xb3 = np.array(x, np.float32, copy=True)  # b3 pre-added to odd columns
    xb3[:, 1::2] += np.asarray(b3, np.float32)
    in_maps = []
    for c in range(NCORES):
        sl = slice(c * BPC, (c + 1) * BPC)
        # x tiles in SBUF layout: [p, (chunk c)] = x[chunk*128 + p, c]
        xh = np.ascontiguousarray(
            xb3[sl]
            .reshape(BPC // 128, 128, D)
            .transpose(1, 0, 2)
            .reshape(128, (BPC // 128) * D)
        )
        # mT tiles in SBUF layout, per variable-width tile t:
        # [p, (t j c)] = x[r0_t + c, 2*(j*128+p)]
        mq = x[sl, 0::2].T.astype(f8)  # [F, BPC]
        blocks = []
        r0 = 0
        for w in WIDTHS:
            blk = mq[:, r0 : r0 + w].reshape(4, 128, w).transpose(1, 0, 2)
            blocks.append(blk.reshape(128, 4 * w))
            r0 += w
        mh = np.ascontiguousarray(np.concatenate(blocks, axis=1))
        in_maps.append(dict(common, x=xh, mT=mh))
    return in_maps


def kernel(x, W1, b1, W2, b2, W3, b3):
    from concourse.bass_utils import run_bass_kernel_spmd

    nc = _get()
    res = run_bass_kernel_spmd(
        nc, _in_maps(x, W1, b1, W2, b2, W3, b3), core_ids=list(range(NCORES))
    )
    return np.concatenate([res.results[c]["y"] for c in range(NCORES)], axis=0)
